# revision 1
# baseline (speedup 1.0000x reference)
"""Coord2HeatmapNet Trainium2 kernel.

out[b,c,j,i] = 10*exp(-(((i+.5)/128 - x)^2 + ((j+.5)/128 - y)^2) / (2*(2/128)^2))

Key facts exploited:
  * Separable: out = fy[j] * fx[i] (outer product per (b,c) heatmap).
  * fp32 exp underflows to exactly 0 beyond ~29 px from the peak, so only a
    64-row window per heatmap is ever nonzero; the rest of the (pre-zeroed)
    output buffer is left untouched.
  * Derivative_Erf activation = 2/sqrt(pi)*exp(-t^2) -> one ScalarE op per
    gaussian factor.
  * Two 64-row windows are packed per 128-partition tile (pairing), halving
    all per-element engine costs.
  * Per-pair indirect scatter DMA writes the 64-row window (128 partitions x
    512B contiguous chunks) at data-dependent offsets computed on-device.

Sharding: pure data parallel, 8 batches per core across 8 NeuronCores.
"""
import sys

for _p in ("/opt/trn_rl_repo", "/root/.axon_site", "/root/.axon_site/_ro/trn_rl_repo",
           "/root/.axon_site/_ro/pypackages"):
    if _p not in sys.path:
        sys.path.append(_p)

import numpy as np

# ---- problem constants (hardcoded per contest contract) ----
S = 128
NUM_CLASS = 68
B_TOTAL = 64
N_CORES = 8
B_LOC = B_TOTAL // N_CORES            # 8 batches per core
NHM = B_LOC * NUM_CLASS               # 544 heatmaps per core
NPAIR = NHM // 2                      # 272
NP = 17                               # pairs per block
NBLK = NPAIR // NP                    # 16 blocks
SIGMA = 2.0 / S
DENOM = 2.0 * SIGMA * SIGMA           # 1/2048
SINV = float(np.sqrt(1.0 / DENOM))    # 45.254834
A = SINV / S                          # grid step in t units
AMP = float(10.0 * np.pi / 4.0)       # (2/sqrt(pi))^2 comp + *10
OUT_ELEMS = NHM * S * S               # flat per-core output

_cache = {}


def _build(trace_scopes=False):
    import concourse.bass as bass
    import concourse.tile as tile
    from concourse import bacc, mybir
    from concourse.bass import IndirectOffsetOnAxis
    from concourse.bass_types import AP

    f32 = mybir.dt.float32
    nc = bacc.Bacc("TRN2", target_bir_lowering=False, debug=False,
                   num_devices=N_CORES)

    coords = nc.dram_tensor("coords", [B_LOC, 2 * NUM_CLASS], f32,
                            kind="ExternalInput")
    out = nc.dram_tensor("out", [OUT_ELEMS], f32, kind="ExternalOutput")
    o2d = out.ap().rearrange("(a b) -> a b", b=1)

    derf = mybir.ActivationFunctionType.Derivative_Erf
    op = mybir.AluOpType

    with tile.TileContext(nc) as tc:
        with tc.tile_pool(name="tabs", bufs=1) as tp, \
             tc.tile_pool(name="main", bufs=3) as mp, \
             tc.tile_pool(name="argp", bufs=2) as ap_pool:
            # ---------- setup tables (all tiny) ----------
            CTB = tp.tile([128, B_LOC * 2 * NUM_CLASS], f32)     # coords bcast
            src = AP(tensor=coords.ap().tensor, offset=0,
                     ap=[[0, 128], [1, B_LOC * 2 * NUM_CLASS]])
            nc.sync.dma_start(CTB[:, :], src)
            ctbv = CTB[:, :].rearrange("p (b c t) -> p (b c) t", t=2,
                                       c=NUM_CLASS)
            XV = ctbv[:, :, 0]                                    # (128,544)
            YV = ctbv[:, :, 1]

            PMOD = tp.tile([128, 1], f32)                         # p mod 64
            nc.gpsimd.iota(PMOD[:], pattern=[[0, 1]], base=0,
                           channel_multiplier=1,
                           allow_small_or_imprecise_dtypes=True)
            nc.vector.tensor_scalar_sub(PMOD[64:128, :], PMOD[64:128, :], 64.0)
            APM = tp.tile([128, 1], f32)                          # a*(p%64)
            nc.vector.tensor_scalar_mul(APM[:], PMOD[:], A)

            IOTA_F = tp.tile([128, S], f32)                       # i (free)
            nc.gpsimd.iota(IOTA_F[:], pattern=[[1, S]], base=0,
                           channel_multiplier=0,
                           allow_small_or_imprecise_dtypes=True)
            ALIN = tp.tile([128, S], f32)                         # a*i + a/2
            nc.vector.tensor_scalar(ALIN[:], IOTA_F[:], A, A * 0.5,
                                    op.mult, op.add)

            SX = tp.tile([128, NHM], f32)                         # s*x
            nc.vector.tensor_scalar_mul(SX[:], XV, SINV)
            SY = tp.tile([128, NHM], f32)                         # s*y
            nc.vector.tensor_scalar_mul(SY[:], YV, SINV)

            # jo = clamp(rint(128*y) - 32, 0, 64)
            CY = tp.tile([128, NHM], f32)
            nc.vector.tensor_scalar_mul(CY[:], YV, float(S))
            CYI = tp.tile([128, NHM], mybir.dt.int32)
            nc.vector.tensor_copy(CYI[:], CY[:])                  # rint
            JOF = tp.tile([128, NHM], f32)
            nc.vector.tensor_copy(JOF[:], CYI[:])                 # back to f32
            nc.vector.tensor_scalar_sub(JOF[:], JOF[:], 32.0)
            nc.vector.tensor_scalar(JOF[:], JOF[:], 0.0, 64.0, op.max, op.min)

            # y-factor argument const per hm: a*jo + a/2 - s*y
            YJO = tp.tile([128, NHM], f32)
            nc.vector.tensor_scalar(YJO[:], JOF[:], A, A * 0.5, op.mult, op.add)
            nc.vector.tensor_sub(YJO[:], YJO[:], SY[:])

            # scatter element offsets: k*16384 + jo*128 (+128*(p%64) later)
            KI = tp.tile([128, NHM], f32)
            nc.gpsimd.iota(KI[:], pattern=[[1, NHM]], base=0,
                           channel_multiplier=0,
                           allow_small_or_imprecise_dtypes=True)
            BASE = tp.tile([128, NHM], f32)
            nc.vector.tensor_scalar_mul(BASE[:], KI[:], float(S * S))
            nc.vector.tensor_scalar(JOF[:], JOF[:], float(S), 0.0,
                                    op.mult, op.add)              # jo*128
            nc.vector.tensor_add(BASE[:], BASE[:], JOF[:])

            # ---------- pair-select tables: rows 0:64 <- even hm, 64:128 <- odd
            def pairsel(dst, srcT):
                sv = srcT[:, :].rearrange("p (q t) -> p q t", t=2)
                nc.vector.tensor_copy(dst[0:64, :], sv[0:64, :, 0])
                nc.vector.tensor_copy(dst[64:128, :], sv[64:128, :, 1])

            SXP = tp.tile([128, NPAIR], f32)
            pairsel(SXP, SX)
            BXP = tp.tile([128, NPAIR], f32)                      # a/2 - s*x
            nc.vector.tensor_scalar(BXP[:], SXP[:], -1.0, A * 0.5,
                                    op.mult, op.add)
            YJP = tp.tile([128, NPAIR], f32)
            pairsel(YJP, YJO)
            FYW = tp.tile([128, NPAIR], f32)                      # y factor col
            nc.scalar.activation(FYW[:], YJP[:], derf, bias=APM[:, 0:1],
                                 scale=1.0)
            nc.vector.tensor_scalar_mul(FYW[:], FYW[:], AMP)

            BP = tp.tile([128, NPAIR], f32)
            pairsel(BP, BASE)
            OFFF = tp.tile([128, NPAIR], f32)
            rowoff = tp.tile([128, 1], f32)
            nc.vector.tensor_scalar_mul(rowoff[:], PMOD[:], float(S))
            nc.vector.tensor_tensor(
                OFFF[:], BP[:],
                AP(tensor=rowoff[:, :].tensor, offset=rowoff[:, :].offset,
                   ap=[[rowoff[:, :].ap[0][0], 128], [0, NPAIR]]),
                op.add)
            OFFI = tp.tile([128, NPAIR], mybir.dt.int32)
            nc.vector.tensor_copy(OFFI[:], OFFF[:])

            # ---------- main loop ----------
            salin = ALIN[:, :]
            for blk in range(NBLK):
                pr0 = blk * NP
                G = mp.tile([128, NP * S], f32, tag="g")
                path_a = (blk % 2 == 0)
                if path_a:
                    # per-pair ACT: G = derf(a*i + (a/2 - s*x))
                    for q in range(NP):
                        nc.scalar.activation(
                            G[:, q * S:(q + 1) * S], IOTA_F[:], derf,
                            bias=BXP[:, pr0 + q:pr0 + q + 1], scale=A)
                else:
                    # block: ARG = ALIN - s*x  (stride-0 bcasts), then derf
                    ARG = ap_pool.tile([128, NP * S], f32, tag="arg")
                    in0 = AP(tensor=salin.tensor, offset=salin.offset,
                             ap=[[salin.ap[0][0], 128], [0, NP], [1, S]])
                    sxp = SXP[:, pr0:pr0 + NP]
                    in1 = AP(tensor=sxp.tensor, offset=sxp.offset,
                             ap=[[sxp.ap[0][0], 128], [1, NP], [0, S]])
                    nc.vector.tensor_tensor(ARG[:, :], in0, in1, op.subtract)
                    nc.scalar.activation(G[:, :], ARG[:, :], derf)
                # multiply by y-factor column (2x_2P tensor_scalar) + scatter
                for q in range(NP):
                    gs = G[:, q * S:(q + 1) * S]
                    nc.vector.tensor_scalar_mul(gs, gs,
                                                FYW[:, pr0 + q:pr0 + q + 1])
                    nc.gpsimd.indirect_dma_start(
                        o2d,
                        IndirectOffsetOnAxis(
                            ap=OFFI[:, pr0 + q:pr0 + q + 1], axis=0),
                        gs, None)

    nc.compile()
    return nc


def _get_nc():
    if "nc" not in _cache:
        _cache["nc"] = _build()
    return _cache["nc"]


def _run(coords_full, trace=False):
    from concourse.bass_utils import run_bass_kernel_spmd

    coords_full = np.ascontiguousarray(np.asarray(coords_full, dtype=np.float32))
    assert coords_full.shape == (B_TOTAL, 2 * NUM_CLASS)
    nc = _get_nc()
    in_maps = [{"coords": coords_full[i * B_LOC:(i + 1) * B_LOC]}
               for i in range(N_CORES)]
    br = run_bass_kernel_spmd(nc, in_maps, core_ids=list(range(N_CORES)),
                              trace=trace)
    parts = [br.results[i]["out"].reshape(B_LOC, NUM_CLASS, S, S)
             for i in range(N_CORES)]
    full = np.concatenate(parts, axis=0)
    return full, br


def kernel(coords):
    return _run(coords, trace=False)[0]


# revision 2
# speedup vs baseline: 8.8244x; 8.8244x over previous
"""Coord2HeatmapNet Trainium2 kernel.

out[b,c,j,i] = 10*exp(-(((i+.5)/128 - x)^2 + ((j+.5)/128 - y)^2) / (2*(2/128)^2))

Exploited structure:
  * Separable: each heatmap = fy[j] (x) fx[i] outer product.
  * fp32 exp underflows to exactly 0 beyond ~29 px from the peak -> only a
    64-row window per heatmap is nonzero; the pre-zeroed output buffer keeps
    the rest at 0.
  * Derivative_Erf activation = 2/sqrt(pi)*exp(-t^2): one ScalarE op per
    gaussian factor vector.
  * Layout: one heatmap per PARTITION. Partition p of group g holds the whole
    64x128 window of heatmap k=g*128+p as 8192 contiguous floats. The outer
    product is one DVE tensor_tensor with stride-0 broadcasts; the write-out
    is ONE indirect scatter DMA per group (one offset per partition, 32KB
    contiguous per heatmap at its data-dependent window position).
  * coords flat index of x_k is 2k (affine), so per-partition coords load is
    a plain strided DMA.

Sharding: pure data parallel, 8 batches per core across 8 NeuronCores.
"""
import sys

for _p in ("/opt/trn_rl_repo", "/root/.axon_site", "/root/.axon_site/_ro/trn_rl_repo",
           "/root/.axon_site/_ro/pypackages"):
    if _p not in sys.path:
        sys.path.append(_p)

import numpy as np

S = 128
NUM_CLASS = 68
B_TOTAL = 64
N_CORES = 8
B_LOC = B_TOTAL // N_CORES            # 8 batches per core
NHM = B_LOC * NUM_CLASS               # 544 heatmaps per core
WIN = 64                              # window rows per heatmap
NG_FULL = NHM // 128                  # 4 full groups of 128 heatmaps
NG_REM = NHM - NG_FULL * 128          # 32 in the last group
GROUPS = [128] * NG_FULL + ([NG_REM] if NG_REM else [])
FREE = WIN * S                        # 8192 elems (32KB) per heatmap window
SIGMA = 2.0 / S
DENOM = 2.0 * SIGMA * SIGMA           # 1/2048
SINV = float(np.sqrt(1.0 / DENOM))    # 45.254834
A = SINV / S
AMP = float(10.0 * np.pi / 4.0)
OUT_ELEMS = NHM * S * S
RCH = 2                               # DVE product ops per group (r-chunks)

_cache = {}


def _build():
    import concourse.bass as bass
    import concourse.tile as tile
    from concourse import bacc, mybir
    from concourse.bass import IndirectOffsetOnAxis
    from concourse.bass_types import AP

    f32 = mybir.dt.float32
    nc = bacc.Bacc("TRN2", target_bir_lowering=False, debug=False,
                   num_devices=N_CORES)

    coords = nc.dram_tensor("coords", [B_LOC, 2 * NUM_CLASS], f32,
                            kind="ExternalInput")
    out = nc.dram_tensor("out", [OUT_ELEMS], f32, kind="ExternalOutput")
    o2d = out.ap().rearrange("(a b) -> a b", b=1)
    cflat = coords.ap().rearrange("b f -> (b f)")

    derf = mybir.ActivationFunctionType.Derivative_Erf
    op = mybir.AluOpType
    NG = len(GROUPS)

    with tile.TileContext(nc) as tc:
        with tc.tile_pool(name="tabs", bufs=1) as tp, \
             tc.tile_pool(name="main", bufs=2) as mp, \
             tc.tile_pool(name="vecs", bufs=2) as vp:
            # ---- per-heatmap coord tables, partition p = heatmap g*128+p ----
            X2 = tp.tile([128, NG], f32)
            Y2 = tp.tile([128, NG], f32)
            for (t, off) in ((X2, 0), (Y2, 1)):
                # full groups: coords_flat[2*(g*128+p) + off]
                src = AP(tensor=cflat.tensor, offset=off,
                         ap=[[2, 128], [256, NG_FULL]])
                nc.sync.dma_start(t[:, 0:NG_FULL], src)
                if NG_REM:
                    srcr = AP(tensor=cflat.tensor,
                              offset=off + 2 * 128 * NG_FULL,
                              ap=[[2, NG_REM], [256, 1]])
                    nc.sync.dma_start(t[0:NG_REM, NG_FULL:NG], srcr)

            # bias for fx: a/2 - s*x
            BX2 = tp.tile([128, NG], f32)
            nc.vector.tensor_scalar(BX2[:], X2[:], -SINV, A * 0.5,
                                    op.mult, op.add)
            # jo = clamp(rint(128*y) - 32, 0, 64)
            JO2 = tp.tile([128, NG], f32)
            nc.vector.tensor_scalar_mul(JO2[:], Y2[:], float(S))
            JO2I = tp.tile([128, NG], mybir.dt.int32)
            nc.vector.tensor_copy(JO2I[:], JO2[:])
            nc.vector.tensor_copy(JO2[:], JO2I[:])
            nc.vector.tensor_scalar_sub(JO2[:], JO2[:], 32.0)
            nc.vector.tensor_scalar(JO2[:], JO2[:], 0.0, 64.0, op.max, op.min)
            # bias for fy: a*jo + a/2 - s*y
            BY2 = tp.tile([128, NG], f32)
            nc.vector.tensor_scalar(BY2[:], Y2[:], -SINV, A * 0.5,
                                    op.mult, op.add)
            T1 = tp.tile([128, NG], f32)
            nc.vector.tensor_scalar_mul(T1[:], JO2[:], A)
            nc.vector.tensor_add(BY2[:], BY2[:], T1[:])
            # scatter offsets: k*16384 + jo*128
            KI2 = tp.tile([128, NG], f32)
            nc.gpsimd.iota(KI2[:], pattern=[[128, NG]], base=0,
                           channel_multiplier=1,
                           allow_small_or_imprecise_dtypes=True)
            OFF2 = tp.tile([128, NG], f32)
            nc.vector.tensor_scalar_mul(OFF2[:], KI2[:], float(S * S))
            nc.vector.tensor_scalar_mul(T1[:], JO2[:], float(S))
            nc.vector.tensor_add(OFF2[:], OFF2[:], T1[:])
            OFF2I = tp.tile([128, NG], mybir.dt.int32)
            nc.vector.tensor_copy(OFF2I[:], OFF2[:])

            IOTA_I = tp.tile([128, S], f32)
            nc.gpsimd.iota(IOTA_I[:], pattern=[[1, S]], base=0,
                           channel_multiplier=0,
                           allow_small_or_imprecise_dtypes=True)
            RIOTA = tp.tile([128, WIN], f32)
            nc.gpsimd.iota(RIOTA[:], pattern=[[1, WIN]], base=0,
                           channel_multiplier=0,
                           allow_small_or_imprecise_dtypes=True)

            # ---- main loop: one group of <=128 heatmaps per iteration ----
            for g, n in enumerate(GROUPS):
                FX = vp.tile([128, S], f32, tag="fx")      # fx row per hm
                nc.scalar.activation(FX[0:n, :], IOTA_I[0:n, :], derf,
                                     bias=BX2[0:n, g:g + 1], scale=A)
                FY = vp.tile([128, WIN], f32, tag="fy")    # fy row per hm
                nc.scalar.activation(FY[0:n, :], RIOTA[0:n, :], derf,
                                     bias=BY2[0:n, g:g + 1], scale=A)
                nc.vector.tensor_scalar_mul(FY[0:n, :], FY[0:n, :], AMP)

                G = mp.tile([128, FREE], f32, tag="g")
                fyap = FY[0:n, :]
                fxap = FX[0:n, :]
                rc = WIN // RCH
                for r in range(RCH):
                    in0 = AP(tensor=fyap.tensor,
                             offset=fyap.offset + r * rc,
                             ap=[[fyap.ap[0][0], n], [1, rc], [0, S]])
                    in1 = AP(tensor=fxap.tensor, offset=fxap.offset,
                             ap=[[fxap.ap[0][0], n], [0, rc], [1, S]])
                    nc.vector.tensor_tensor(
                        G[0:n, r * rc * S:(r + 1) * rc * S], in0, in1,
                        op.mult)
                nc.gpsimd.indirect_dma_start(
                    o2d,
                    IndirectOffsetOnAxis(ap=OFF2I[0:n, g:g + 1], axis=0),
                    G[0:n, :], None)

    nc.compile()
    return nc


def _get_nc():
    if "nc" not in _cache:
        _cache["nc"] = _build()
    return _cache["nc"]


def _run(coords_full, trace=False):
    from concourse.bass_utils import run_bass_kernel_spmd

    coords_full = np.ascontiguousarray(np.asarray(coords_full, dtype=np.float32))
    assert coords_full.shape == (B_TOTAL, 2 * NUM_CLASS)
    nc = _get_nc()
    in_maps = [{"coords": coords_full[i * B_LOC:(i + 1) * B_LOC]}
               for i in range(N_CORES)]
    br = run_bass_kernel_spmd(nc, in_maps, core_ids=list(range(N_CORES)),
                              trace=trace)
    parts = [br.results[i]["out"].reshape(B_LOC, NUM_CLASS, S, S)
             for i in range(N_CORES)]
    full = np.concatenate(parts, axis=0)
    return full, br


def kernel(coords):
    return _run(coords, trace=False)[0]
